# revision 13
# baseline (speedup 1.0000x reference)
"""DeformConv1d (modulated, K=3, stride=1, pad=1, dil=1) on 8 Trainium2
NeuronCores via Bass.

Contract: kernel(**inputs) takes the FULL inputs
  x[16,256,4096] f32, weight[256,256,3] f32, offset[16,3,4096] f32,
  mask[16,3,4096] f32, bias[256] f32
and returns the full output [16,256,4096] f32.

Strategy (data-parallel over batch, 2 batches per core):
  out[b,o,w] = sum_k m[k,w]*(w0*z_k[o,i0] + w1*z_k[o,i0+1]) + bias[o],
  z_k = W_k @ x[b].
  On device, z_k is produced TRANSPOSED ([w,oc] tiles) by matmuls with
  x-slices stationary, staged to DRAM, then two indirect-DMA row gathers
  per tap fetch z_k[i0]/z_k[i0+1]; VectorE applies the interpolation
  weights (precomputed host-side from offset/mask, along with clamped
  indices) and the bias. The output leaves the device transposed and is
  unpermuted on the host.
"""
import numpy as np

import concourse.bass as bass
import concourse.bacc as bacc
import concourse.tile as tile
from concourse import mybir
from concourse.bass_utils import run_bass_kernel_spmd

F32 = mybir.dt.float32
F32R = mybir.dt.float32r
BF16 = mybir.dt.bfloat16
I32 = mybir.dt.int32

B2 = 2          # batches per core
K = 3
W = 4096
NT = W // 128   # 32 w-tiles
N_CORES = 8


def _build(reps: int = 1, fast: bool = False):
    nc = bacc.Bacc("TRN2", target_bir_lowering=False, debug=False)

    z_dt = F32  # keep staging/gather fp32 for precision; fast only switches matmul to fp32r

    x_in = nc.dram_tensor("x_in", [B2, 128, 2, W], F32, kind="ExternalInput")
    wT_in = nc.dram_tensor("wT_in", [128, K, 2, 256], F32, kind="ExternalInput")
    bias_in = nc.dram_tensor("bias_in", [128, 256], F32, kind="ExternalInput")
    idx0_in = nc.dram_tensor("idx0_in", [B2, 128, K, NT], I32, kind="ExternalInput")
    idx1_in = nc.dram_tensor("idx1_in", [B2, 128, K, NT], I32, kind="ExternalInput")
    c0_in = nc.dram_tensor("c0_in", [B2, 128, K, NT], F32, kind="ExternalInput")
    c1_in = nc.dram_tensor("c1_in", [B2, 128, K, NT], F32, kind="ExternalInput")
    outT = nc.dram_tensor("outT", [B2, 128, NT, 256], F32, kind="ExternalOutput")
    zTs = [[nc.dram_tensor(f"zT_{b}_{k}", [W, 256], z_dt) for k in range(K)]
           for b in range(B2)]

    with tile.TileContext(nc) as tc:
        with (
            tc.tile_pool(name="const", bufs=1) as cpool,
            tc.tile_pool(name="xp", bufs=1) as xpool,
            tc.tile_pool(name="zstage", bufs=6) as zpool,
            tc.tile_pool(name="gp", bufs=1) as gpool,
            tc.tile_pool(name="coefp", bufs=2) as coefpool,
            tc.tile_pool(name="interp", bufs=2) as ipool,
            tc.tile_pool(name="accp", bufs=1) as apool,
            tc.tile_pool(name="psum", bufs=8, space="PSUM") as psum,
        ):
            w_raw = cpool.tile([128, K, 2, 256], F32, tag="wraw")
            nc.sync.dma_start(out=w_raw[:], in_=wT_in[:])
            if fast:
                w_rt = cpool.tile([128, K, 2, 256], F32R, tag="wr")
                nc.vector.tensor_copy(w_rt[:], w_raw[:])
                w_r = w_rt[:]
            else:
                w_r = w_raw[:]
            bias_sb = cpool.tile([128, 256], F32, tag="bias")
            nc.sync.dma_start(out=bias_sb[:], in_=bias_in[:])
            zero_b = cpool.tile([128, 1], F32, tag="zb")
            nc.gpsimd.memset(zero_b[:], 0.0)

            for rep in range(reps):
                for b in range(B2):
                    x_raw = xpool.tile([128, 2, W], F32, tag="xraw")
                    nc.sync.dma_start(out=x_raw[:], in_=x_in[b])
                    if fast:
                        x_rt = xpool.tile([128, 2, W], F32R, tag="xr")
                        nc.vector.tensor_copy(x_rt[:], x_raw[:])
                        x_r = x_rt[:]
                    else:
                        x_r = x_raw[:]

                    idx0_sb = coefpool.tile([128, K, NT], I32, tag="i0")
                    idx1_sb = coefpool.tile([128, K, NT], I32, tag="i1")
                    c0_sb = coefpool.tile([128, K, NT], F32, tag="c0")
                    c1_sb = coefpool.tile([128, K, NT], F32, tag="c1")
                    nc.sync.dma_start(out=idx0_sb[:], in_=idx0_in[b])
                    nc.sync.dma_start(out=idx1_sb[:], in_=idx1_in[b])
                    nc.sync.dma_start(out=c0_sb[:], in_=c0_in[b])
                    nc.sync.dma_start(out=c1_sb[:], in_=c1_in[b])

                    acc = apool.tile([128, NT, 256], F32, tag="acc")

                    for k in range(K):
                        for wt in range(NT):
                            zp = psum.tile([128, 256], F32, tag="zp")
                            ws = wt * 128
                            for cc in range(2):
                                nc.tensor.matmul(
                                    zp[:],
                                    x_r[:, cc, ws:ws + 128],
                                    w_r[:, k, cc],
                                    start=(cc == 0),
                                    stop=(cc == 1),
                                )
                            zst = zpool.tile([128, 256], z_dt, tag="zst")
                            nc.scalar.activation(
                                zst[:], zp[:],
                                mybir.ActivationFunctionType.Identity,
                                bias=zero_b[:])
                            nc.sync.dma_start(
                                out=zTs[b][k][ws:ws + 128], in_=zst[:])

                        zrows = zTs[b][k][:]  # [4096, 256]
                        H = NT // 2
                        for h in range(2):
                            hs = h * H
                            g0 = gpool.tile([128, H, 256], z_dt, tag="g0")
                            g1 = gpool.tile([128, H, 256], z_dt, tag="g1")
                            for t in range(H):
                                nc.gpsimd.indirect_dma_start(
                                    out=g0[:, t], out_offset=None, in_=zrows,
                                    in_offset=bass.IndirectOffsetOnAxis(
                                        ap=idx0_sb[:, k, hs + t:hs + t + 1],
                                        axis=0))
                                nc.gpsimd.indirect_dma_start(
                                    out=g1[:, t], out_offset=None, in_=zrows,
                                    in_offset=bass.IndirectOffsetOnAxis(
                                        ap=idx1_sb[:, k, hs + t:hs + t + 1],
                                        axis=0))

                            c0b = c0_sb[:, k, hs:hs + H][:, :, None] \
                                .broadcast_to([128, H, 256])
                            c1b = c1_sb[:, k, hs:hs + H][:, :, None] \
                                .broadcast_to([128, H, 256])
                            t0 = ipool.tile([128, H, 256], F32, tag="t0")
                            acch = acc[:, hs:hs + H]
                            nc.vector.tensor_tensor(t0[:], g0[:], c0b,
                                                    mybir.AluOpType.mult)
                            if k == 0:
                                biasb = bias_sb[:][:, None, :].broadcast_to(
                                    [128, H, 256])
                                nc.vector.tensor_tensor(
                                    acch, t0[:], biasb, mybir.AluOpType.add)
                            else:
                                nc.vector.tensor_tensor(
                                    acch, acch, t0[:], mybir.AluOpType.add)
                            t1 = ipool.tile([128, H, 256], F32, tag="t0")
                            nc.gpsimd.tensor_tensor(t1[:], g1[:], c1b,
                                                    mybir.AluOpType.mult)
                            nc.vector.tensor_tensor(
                                acch, acch, t1[:], mybir.AluOpType.add)

                    nc.sync.dma_start(out=outT[b], in_=acc[:])

    nc.compile()
    return nc


def _prep_coeffs(offset, mask):
    """offset/mask [B,K,W] -> idx0,c0,idx1,c1 in [B,128,K,NT] device layout,
    slot (p,t) <-> w = t*128+p. Float op order replicates the reference."""
    B = offset.shape[0]
    base = np.arange(W, dtype=np.float32) * np.float32(1.0) - np.float32(1.0)
    kpos = np.arange(K, dtype=np.float32) * np.float32(1.0)
    bk = (base[None, :] + kpos[:, None]).astype(np.float32)
    p = (bk[None] + offset).astype(np.float32)
    i0f = np.floor(p)
    w1 = (p - i0f).astype(np.float32)
    w0 = (np.float32(1.0) - w1).astype(np.float32)
    i0 = i0f.astype(np.int64)
    i1 = i0 + 1
    v0 = (i0 >= 0) & (i0 < W)
    v1 = (i1 >= 0) & (i1 < W)
    c0 = (mask * w0 * v0).astype(np.float32)
    c1 = (mask * w1 * v1).astype(np.float32)
    idx0 = np.clip(i0, 0, W - 1).astype(np.int32)
    idx1 = np.clip(i1, 0, W - 1).astype(np.int32)

    def lay(a):
        return np.ascontiguousarray(a.reshape(B, K, NT, 128).transpose(0, 3, 1, 2))

    return lay(idx0), lay(c0), lay(idx1), lay(c1)


def _core_inputs(x, weight, offset, mask, bias, core):
    b0 = 2 * core
    idx0, c0, idx1, c1 = _prep_coeffs(offset[b0:b0 + 2], mask[b0:b0 + 2])
    OC = weight.shape[0]
    return {
        "x_in": np.ascontiguousarray(
            x[b0:b0 + 2].reshape(2, 2, 128, W).transpose(0, 2, 1, 3)
        ).astype(np.float32),
        "wT_in": np.ascontiguousarray(
            weight.transpose(2, 1, 0).reshape(K, 2, 128, OC)
            .transpose(2, 0, 1, 3)).astype(np.float32),
        "bias_in": np.ascontiguousarray(
            np.broadcast_to(bias.reshape(1, -1), (128, OC))).astype(np.float32),
        "idx0_in": idx0, "idx1_in": idx1, "c0_in": c0, "c1_in": c1,
    }


_NC_CACHE = {}


def _get_nc(reps=1, fast=False):
    key = (reps, fast)
    if key not in _NC_CACHE:
        _NC_CACHE[key] = _build(reps=reps, fast=fast)
    return _NC_CACHE[key]


_DISPATCH = None


def _get_dispatch(nc):
    """Build (once) a cached jitted shard_map dispatcher over 8 cores,
    mirroring bass2jax.run_bass_via_pjrt but without per-call retracing."""
    global _DISPATCH
    if _DISPATCH is not None:
        return _DISPATCH
    import jax
    from jax.sharding import Mesh, PartitionSpec
    from jax.experimental.shard_map import shard_map
    from concourse import bass2jax, mybir as mb
    bass2jax.install_neuronx_cc_hook()

    partition_name = (nc.partition_id_tensor.name
                      if nc.partition_id_tensor else None)
    in_names, out_names, out_avals, zero_outs = [], [], [], []
    for alloc in nc.m.functions[0].allocations:
        if not isinstance(alloc, mb.MemoryLocationSet):
            continue
        name = alloc.memorylocations[0].name
        if alloc.kind == "ExternalInput":
            if name != partition_name:
                in_names.append(name)
        elif alloc.kind == "ExternalOutput":
            shape = tuple(alloc.tensor_shape)
            dtype = mb.dt.np(alloc.dtype)
            out_names.append(name)
            out_avals.append(jax.core.ShapedArray(shape, dtype))
            zero_outs.append(np.zeros(shape, dtype))
    n_params = len(in_names)
    n_outs = len(out_avals)
    all_in_names = list(in_names) + list(out_names)
    if partition_name is not None:
        all_in_names.append(partition_name)

    def _body(*args):
        operands = list(args)
        if partition_name is not None:
            operands.append(bass2jax.partition_id_tensor())
        outs = bass2jax._bass_exec_p.bind(
            *operands,
            out_avals=tuple(out_avals),
            in_names=tuple(all_in_names),
            out_names=tuple(out_names),
            lowering_input_output_aliases=(),
            sim_require_finite=True,
            sim_require_nnan=True,
            nc=nc,
        )
        return tuple(outs)

    devices = jax.devices()[:N_CORES]
    mesh = Mesh(np.asarray(devices), ("core",))
    in_specs = (PartitionSpec("core"),) * (n_params + n_outs)
    out_specs = (PartitionSpec("core"),) * n_outs
    donate = tuple(range(n_params, n_params + n_outs))
    sharded = jax.jit(
        shard_map(_body, mesh=mesh, in_specs=in_specs, out_specs=out_specs,
                  check_rep=False),
        donate_argnums=donate, keep_unused=True)
    _DISPATCH = (sharded, in_names, out_names, out_avals, zero_outs)
    return _DISPATCH


def kernel(x, weight, offset, mask, bias):
    x = np.asarray(x, dtype=np.float32)
    weight = np.asarray(weight, dtype=np.float32)
    offset = np.asarray(offset, dtype=np.float32)
    mask = np.asarray(mask, dtype=np.float32)
    bias = np.asarray(bias, dtype=np.float32)

    nc = _get_nc(fast=True)
    sharded, in_names, out_names, out_avals, zero_outs = _get_dispatch(nc)
    ins_list = [_core_inputs(x, weight, offset, mask, bias, core)
                for core in range(N_CORES)]
    concat_in = [np.concatenate([ins_list[c][n] for c in range(N_CORES)],
                                axis=0) for n in in_names]
    concat_zeros = [np.zeros((N_CORES * z.shape[0], *z.shape[1:]), z.dtype)
                    for z in zero_outs]
    out_arrs = sharded(*concat_in, *concat_zeros)
    i = out_names.index("outT")
    allT = np.asarray(out_arrs[i]).reshape(N_CORES, *out_avals[i].shape)

    out = np.empty((16, 256, W), np.float32)
    for core in range(N_CORES):
        out[2 * core:2 * core + 2] = np.ascontiguousarray(
            allT[core].transpose(0, 3, 2, 1).reshape(2, 256, W))
    return out


# revision 16
# speedup vs baseline: 783.8375x; 783.8375x over previous
"""DeformConv1d (modulated, K=3, stride=1, pad=1, dil=1) on 8 Trainium2
NeuronCores via Bass.

Contract: kernel(**inputs) takes the FULL inputs
  x[16,256,4096] f32, weight[256,256,3] f32, offset[16,3,4096] f32,
  mask[16,3,4096] f32, bias[256] f32
and returns the full output [16,256,4096] f32.

Strategy (data-parallel over batch, 2 batches per core):
  out[b,o,w] = sum_k m[k,w]*(w0*z_k[o,i0] + w1*z_k[o,i0+1]) + bias[o],
  z_k = W_k @ x[b].
  On device, z_k is produced TRANSPOSED ([w,oc] tiles) by matmuls with
  x-slices stationary, staged to DRAM, then two indirect-DMA row gathers
  per tap fetch z_k[i0]/z_k[i0+1]; VectorE applies the interpolation
  weights (precomputed host-side from offset/mask, along with clamped
  indices) and the bias. The output leaves the device transposed and is
  unpermuted on the host.
"""
import numpy as np

import concourse.bass as bass
import concourse.bacc as bacc
import concourse.tile as tile
from concourse import mybir
from concourse.bass_utils import run_bass_kernel_spmd

F32 = mybir.dt.float32
F32R = mybir.dt.float32r
BF16 = mybir.dt.bfloat16
I32 = mybir.dt.int32

B2 = 2          # batches per core
K = 3
W = 4096
NT = W // 128   # 32 w-tiles
N_CORES = 8


def _build(reps: int = 1, fast: bool = False):
    nc = bacc.Bacc("TRN2", target_bir_lowering=False, debug=False)

    z_dt = F32  # keep staging/gather fp32 for precision; fast only switches matmul to fp32r

    x_in = nc.dram_tensor("x_in", [B2, 128, 2, W], F32, kind="ExternalInput")
    wT_in = nc.dram_tensor("wT_in", [128, K, 2, 256], F32, kind="ExternalInput")
    bias_in = nc.dram_tensor("bias_in", [128, 256], F32, kind="ExternalInput")
    idx0_in = nc.dram_tensor("idx0_in", [B2, 128, K, NT], I32, kind="ExternalInput")
    idx1_in = nc.dram_tensor("idx1_in", [B2, 128, K, NT], I32, kind="ExternalInput")
    c0_in = nc.dram_tensor("c0_in", [B2, 128, K, NT], F32, kind="ExternalInput")
    c1_in = nc.dram_tensor("c1_in", [B2, 128, K, NT], F32, kind="ExternalInput")
    outT = nc.dram_tensor("outT", [B2, 128, NT, 256], F32, kind="ExternalOutput")
    zTs = [[nc.dram_tensor(f"zT_{b}_{k}", [W, 256], z_dt) for k in range(K)]
           for b in range(B2)]

    with tile.TileContext(nc) as tc:
        with (
            tc.tile_pool(name="const", bufs=1) as cpool,
            tc.tile_pool(name="xp", bufs=1) as xpool,
            tc.tile_pool(name="zstage", bufs=6) as zpool,
            tc.tile_pool(name="gp", bufs=1) as gpool,
            tc.tile_pool(name="coefp", bufs=2) as coefpool,
            tc.tile_pool(name="interp", bufs=2) as ipool,
            tc.tile_pool(name="accp", bufs=1) as apool,
            tc.tile_pool(name="psum", bufs=8, space="PSUM") as psum,
        ):
            w_raw = cpool.tile([128, K, 2, 256], F32, tag="wraw")
            nc.sync.dma_start(out=w_raw[:], in_=wT_in[:])
            if fast:
                w_rt = cpool.tile([128, K, 2, 256], F32R, tag="wr")
                nc.vector.tensor_copy(w_rt[:], w_raw[:])
                w_r = w_rt[:]
            else:
                w_r = w_raw[:]
            bias_sb = cpool.tile([128, 256], F32, tag="bias")
            nc.sync.dma_start(out=bias_sb[:], in_=bias_in[:])
            zero_b = cpool.tile([128, 1], F32, tag="zb")
            nc.gpsimd.memset(zero_b[:], 0.0)

            for rep in range(reps):
                for b in range(B2):
                    x_raw = xpool.tile([128, 2, W], F32, tag="xraw")
                    nc.sync.dma_start(out=x_raw[:], in_=x_in[b])
                    if fast:
                        x_rt = xpool.tile([128, 2, W], F32R, tag="xr")
                        nc.vector.tensor_copy(x_rt[:], x_raw[:])
                        x_r = x_rt[:]
                    else:
                        x_r = x_raw[:]

                    idx0_sb = coefpool.tile([128, K, NT], I32, tag="i0")
                    idx1_sb = coefpool.tile([128, K, NT], I32, tag="i1")
                    c0_sb = coefpool.tile([128, K, NT], F32, tag="c0")
                    c1_sb = coefpool.tile([128, K, NT], F32, tag="c1")
                    nc.sync.dma_start(out=idx0_sb[:], in_=idx0_in[b])
                    nc.sync.dma_start(out=idx1_sb[:], in_=idx1_in[b])
                    nc.sync.dma_start(out=c0_sb[:], in_=c0_in[b])
                    nc.sync.dma_start(out=c1_sb[:], in_=c1_in[b])

                    acc = apool.tile([128, NT, 256], F32, tag="acc")

                    for k in range(K):
                        for wt in range(NT):
                            zp = psum.tile([128, 256], F32, tag="zp")
                            ws = wt * 128
                            for cc in range(2):
                                nc.tensor.matmul(
                                    zp[:],
                                    x_r[:, cc, ws:ws + 128],
                                    w_r[:, k, cc],
                                    start=(cc == 0),
                                    stop=(cc == 1),
                                )
                            zst = zpool.tile([128, 256], z_dt, tag="zst")
                            nc.scalar.activation(
                                zst[:], zp[:],
                                mybir.ActivationFunctionType.Identity,
                                bias=zero_b[:])
                            nc.sync.dma_start(
                                out=zTs[b][k][ws:ws + 128], in_=zst[:])

                        zrows = zTs[b][k][:]  # [4096, 256]
                        H = NT // 2
                        for h in range(2):
                            hs = h * H
                            g0 = gpool.tile([128, H, 256], z_dt, tag="g0")
                            g1 = gpool.tile([128, H, 256], z_dt, tag="g1")
                            for t in range(H):
                                nc.gpsimd.indirect_dma_start(
                                    out=g0[:, t], out_offset=None, in_=zrows,
                                    in_offset=bass.IndirectOffsetOnAxis(
                                        ap=idx0_sb[:, k, hs + t:hs + t + 1],
                                        axis=0))
                                nc.gpsimd.indirect_dma_start(
                                    out=g1[:, t], out_offset=None, in_=zrows,
                                    in_offset=bass.IndirectOffsetOnAxis(
                                        ap=idx1_sb[:, k, hs + t:hs + t + 1],
                                        axis=0))

                            c0b = c0_sb[:, k, hs:hs + H][:, :, None] \
                                .broadcast_to([128, H, 256])
                            c1b = c1_sb[:, k, hs:hs + H][:, :, None] \
                                .broadcast_to([128, H, 256])
                            t0 = ipool.tile([128, H, 256], F32, tag="t0")
                            acch = acc[:, hs:hs + H]
                            nc.vector.tensor_tensor(t0[:], g0[:], c0b,
                                                    mybir.AluOpType.mult)
                            if k == 0:
                                biasb = bias_sb[:][:, None, :].broadcast_to(
                                    [128, H, 256])
                                nc.vector.tensor_tensor(
                                    acch, t0[:], biasb, mybir.AluOpType.add)
                            else:
                                nc.vector.tensor_tensor(
                                    acch, acch, t0[:], mybir.AluOpType.add)
                            t1 = ipool.tile([128, H, 256], F32, tag="t0")
                            nc.gpsimd.tensor_tensor(t1[:], g1[:], c1b,
                                                    mybir.AluOpType.mult)
                            nc.vector.tensor_tensor(
                                acch, acch, t1[:], mybir.AluOpType.add)

                    nc.sync.dma_start(out=outT[b], in_=acc[:])

    nc.compile()
    return nc


def _prep_coeffs(offset, mask):
    """offset/mask [B,K,W] -> idx0,c0,idx1,c1 in [B,128,K,NT] device layout,
    slot (p,t) <-> w = t*128+p. Float op order replicates the reference."""
    B = offset.shape[0]
    base = np.arange(W, dtype=np.float32) * np.float32(1.0) - np.float32(1.0)
    kpos = np.arange(K, dtype=np.float32) * np.float32(1.0)
    bk = (base[None, :] + kpos[:, None]).astype(np.float32)
    p = (bk[None] + offset).astype(np.float32)
    i0f = np.floor(p)
    w1 = (p - i0f).astype(np.float32)
    w0 = (np.float32(1.0) - w1).astype(np.float32)
    i0 = i0f.astype(np.int64)
    i1 = i0 + 1
    v0 = (i0 >= 0) & (i0 < W)
    v1 = (i1 >= 0) & (i1 < W)
    c0 = (mask * w0 * v0).astype(np.float32)
    c1 = (mask * w1 * v1).astype(np.float32)
    idx0 = np.clip(i0, 0, W - 1).astype(np.int32)
    idx1 = np.clip(i1, 0, W - 1).astype(np.int32)

    def lay(a):
        return np.ascontiguousarray(a.reshape(B, K, NT, 128).transpose(0, 3, 1, 2))

    return lay(idx0), lay(c0), lay(idx1), lay(c1)


def _core_inputs(x, weight, offset, mask, bias, core):
    b0 = 2 * core
    idx0, c0, idx1, c1 = _prep_coeffs(offset[b0:b0 + 2], mask[b0:b0 + 2])
    OC = weight.shape[0]
    return {
        "x_in": np.ascontiguousarray(
            x[b0:b0 + 2].reshape(2, 2, 128, W).transpose(0, 2, 1, 3)
        ).astype(np.float32),
        "wT_in": np.ascontiguousarray(
            weight.transpose(2, 1, 0).reshape(K, 2, 128, OC)
            .transpose(2, 0, 1, 3)).astype(np.float32),
        "bias_in": np.ascontiguousarray(
            np.broadcast_to(bias.reshape(1, -1), (128, OC))).astype(np.float32),
        "idx0_in": idx0, "idx1_in": idx1, "c0_in": c0, "c1_in": c1,
    }


_NC_CACHE = {}


def _get_nc(reps=1, fast=False):
    key = (reps, fast)
    if key not in _NC_CACHE:
        _NC_CACHE[key] = _build(reps=reps, fast=fast)
    return _NC_CACHE[key]


_DISPATCH = None


def _get_dispatch(nc):
    """Build (once) a cached jitted shard_map dispatcher over 8 cores,
    mirroring bass2jax.run_bass_via_pjrt but without per-call retracing."""
    global _DISPATCH
    if _DISPATCH is not None:
        return _DISPATCH
    import jax
    from jax.sharding import Mesh, PartitionSpec
    from jax.experimental.shard_map import shard_map
    from concourse import bass2jax, mybir as mb
    bass2jax.install_neuronx_cc_hook()

    partition_name = (nc.partition_id_tensor.name
                      if nc.partition_id_tensor else None)
    in_names, out_names, out_avals, zero_outs = [], [], [], []
    for alloc in nc.m.functions[0].allocations:
        if not isinstance(alloc, mb.MemoryLocationSet):
            continue
        name = alloc.memorylocations[0].name
        if alloc.kind == "ExternalInput":
            if name != partition_name:
                in_names.append(name)
        elif alloc.kind == "ExternalOutput":
            shape = tuple(alloc.tensor_shape)
            dtype = mb.dt.np(alloc.dtype)
            out_names.append(name)
            out_avals.append(jax.core.ShapedArray(shape, dtype))
            zero_outs.append(np.zeros(shape, dtype))
    n_params = len(in_names)
    n_outs = len(out_avals)
    all_in_names = list(in_names) + list(out_names)
    if partition_name is not None:
        all_in_names.append(partition_name)

    def _body(*args):
        operands = list(args)
        if partition_name is not None:
            operands.append(bass2jax.partition_id_tensor())
        outs = bass2jax._bass_exec_p.bind(
            *operands,
            out_avals=tuple(out_avals),
            in_names=tuple(all_in_names),
            out_names=tuple(out_names),
            lowering_input_output_aliases=(),
            sim_require_finite=True,
            sim_require_nnan=True,
            nc=nc,
        )
        return tuple(outs)

    devices = jax.devices()[:N_CORES]
    mesh = Mesh(np.asarray(devices), ("core",))
    in_specs = (PartitionSpec("core"),) * (n_params + n_outs)
    out_specs = (PartitionSpec("core"),) * n_outs
    donate = tuple(range(n_params, n_params + n_outs))
    sharded = jax.jit(
        shard_map(_body, mesh=mesh, in_specs=in_specs, out_specs=out_specs,
                  check_rep=False),
        donate_argnums=donate, keep_unused=True)
    _DISPATCH = (sharded, in_names, out_names, out_avals, zero_outs)
    return _DISPATCH


def kernel(x, weight, offset, mask, bias):
    x = np.asarray(x, dtype=np.float32)
    weight = np.asarray(weight, dtype=np.float32)
    offset = np.asarray(offset, dtype=np.float32)
    mask = np.asarray(mask, dtype=np.float32)
    bias = np.asarray(bias, dtype=np.float32)

    nc = _get_nc(fast=True)
    sharded, in_names, out_names, out_avals, zero_outs = _get_dispatch(nc)
    ins_list = [_core_inputs(x, weight, offset, mask, bias, core)
                for core in range(N_CORES)]
    concat_in = [np.concatenate([ins_list[c][n] for c in range(N_CORES)],
                                axis=0) for n in in_names]
    concat_zeros = [np.zeros((N_CORES * z.shape[0], *z.shape[1:]), z.dtype)
                    for z in zero_outs]
    out_arrs = sharded(*concat_in, *concat_zeros)
    i = out_names.index("outT")
    allT = np.asarray(out_arrs[i]).reshape(N_CORES, *out_avals[i].shape)

    out = np.empty((16, 256, W), np.float32)
    for core in range(N_CORES):
        out[2 * core:2 * core + 2] = np.ascontiguousarray(
            allT[core].transpose(0, 3, 2, 1).reshape(2, 256, W))
    return out
